# revision 15
# baseline (speedup 1.0000x reference)
"""Trainium2 Bass kernel for nn_CausalMultimodal (gnn_message_passing).

Math (per batch row b, fully row-local so batch shards freely over 8 cores):
    mask[i,j]  = (matrix*(matrix>0.1))[i,j] > 0.1
    agg[b,d]   = (Z[b,:] @ mask[d,:]) / count[d]   (0 when count==0)
    hidden     = relu(Z[b,d]*W1[d,0,h] + agg[b,d]*W1[d,1,h] + b1[d,h])
    E[b,d]     = sum_h hidden[b,d,h]*W2[d,h] + b2[d]

Since agg = Z @ M2 with M2[j,d] = mask[d,j]/count[d], the first layer folds
into one 32x128 matrix A computed host-side: U[b, 32h+d] = (Z @ A)[b, 32h+d];
then E = W2sel.T @ relu(U + b1) + b2 with W2sel (128,32) block-sparse.

v3 dataflow (PSUM-evacuation-bound; ACT+DVE are the critical engines):
  - Host pre-permutes Z (bf16) into the exact strip layout mm1 streams, and
    un-permutes the strip-layout E output. No DVE transposes on device.
  - Per 2048-row block: 4 row-tiled concurrent MMs (K=32, tile_position
    (32a,0)) write U into TWO (128,1024) PSUM pair-tiles (strips 0-1, 2-3).
    ACT relus pair01 while DVE relus pair23 (independent tiles, different
    banks). 4 col-tiled concurrent MMs (M=32, tile_position (0,32a)) write
    E into a separate (128,512) eps tile; E is evacuated PSUM->SBUF bf16 on
    alternating engines and DMA'd per megatile.
  - PSUM budget: 3 pair-slots x 2 banks + 2 eps banks = 8 banks. Every
    dependency edge has >= 1 block-period of slack (no aliasing), so the
    wall time tracks the busy-bound of the ACT/DVE engines.
  - Emission is software-pipelined: mm1 of block i+1 enters the PE FIFO
    before mm3 of block i; E-evac of block i is emitted one iteration late
    so it never head-of-line-blocks a relu in the strict-FIFO ACT/DVE
    queues.
"""

import os

import ml_dtypes
import numpy as np

import concourse.bacc as bacc
import concourse.tile as tile
from concourse import mybir
from concourse import bass_utils

B_TOTAL, D, H = 1048576, 32, 4
NCORES = 8
R = B_TOTAL // NCORES        # rows per core (131072)
NMT = 16                     # megatiles per core
BLOCKS_PER_MT = 4
NBLK = NMT * BLOCKS_PER_MT   # 64 blocks of 2048 rows
BF16 = ml_dtypes.bfloat16

EEVAC = os.environ.get("NNK_EEVAC", "alt")  # alt | act | dve
EOUT = os.environ.get("NNK_EOUT", "bf16")   # bf16 | f32
ZP_BUFS = int(os.environ.get("NNK_ZPBUFS", "3"))
PAIR_BUFS = int(os.environ.get("NNK_PAIRBUFS", "3"))
EPS_BUFS = int(os.environ.get("NNK_EPSBUFS", "2"))

_module_cache = {}


def _build_module(b1_zero, b2_zero):
    key = (b1_zero, b2_zero, EEVAC, EOUT, ZP_BUFS, PAIR_BUFS, EPS_BUFS)
    if key in _module_cache:
        return _module_cache[key]

    f32 = mybir.dt.float32
    bf = mybir.dt.bfloat16
    edt = bf if EOUT == "bf16" else f32

    nc = bacc.Bacc("TRN2", target_bir_lowering=False, debug=False,
                   num_devices=NCORES)

    ZP = nc.dram_tensor("ZP", (NMT, 128, 2048), bf, kind="ExternalInput").ap()
    # packed consts: CB[p] = [A4 row p (128 bf16) | W2S row p (32 bf16)]
    CB = nc.dram_tensor("CB", (128, 128 + D), bf, kind="ExternalInput").ap()
    # CF[p] = [b1v[p], b2v[p]]
    CF = nc.dram_tensor("CF", (128, 2), f32, kind="ExternalInput").ap()
    EP = nc.dram_tensor("EP", (NMT, 128, 2048), edt, kind="ExternalOutput").ap()

    with tile.TileContext(nc) as tc:
        with (
            tc.tile_pool(name="const", bufs=1) as constp,
            tc.tile_pool(name="zp", bufs=ZP_BUFS) as zpp,
            tc.tile_pool(name="vt", bufs=2) as vtp,
            tc.tile_pool(name="et", bufs=2) as etp,
            tc.tile_pool(name="ug", bufs=PAIR_BUFS, space="PSUM") as ugp,
            tc.tile_pool(name="ep", bufs=EPS_BUFS, space="PSUM") as epp,
        ):
            zpt = {}    # megatile -> SBUF tile
            ugs = {}    # block -> (pair01, pair23) PSUM tiles
            vts = {}    # block -> (vt01, vt23) SBUF tiles
            eps_ = {}   # block -> eps PSUM tile
            ets = {}    # megatile -> SBUF E tile

            def fetch_zp(m):
                t = zpp.tile([128, 2048], bf, tag="zp", name=f"zp{m}")
                nc.sync.dma_start(out=t, in_=ZP[m])
                zpt[m] = t

            # ZP DMAs first so the input stream heads the DMA queues;
            # the packed const DMAs overlap the first megatile transfer.
            for m in range(min(ZP_BUFS - 1, NMT)):
                fetch_zp(m)
            cbt = constp.tile([128, 128 + D], bf, name="cB")
            nc.sync.dma_start(out=cbt, in_=CB)
            cft = constp.tile([128, 2], f32, name="cF")
            nc.sync.dma_start(out=cft, in_=CF)
            acst = cbt[:, 0:128]
            wcst = cbt[:, 128:128 + D]
            b1v = cft[:, 0:1]
            b2v = cft[:, 1:2]

            def mm1(i):
                m, t = divmod(i, BLOCKS_PER_MT)
                if t == 0 and m + ZP_BUFS - 1 < NMT:
                    fetch_zp(m + ZP_BUFS - 1)
                p01 = ugp.tile([128, 1024], f32, tag="ug", name="u01")
                p23 = ugp.tile([128, 1024], f32, tag="ug", name="u23")
                z = zpt[m]
                for a in range(4):
                    dst = p01 if a < 2 else p23
                    nc.tensor.matmul(
                        dst[:, 512 * (a % 2):512 * (a % 2 + 1)],
                        lhsT=acst[32 * a:32 * (a + 1), :],
                        rhs=z[32 * a:32 * (a + 1), 512 * t:512 * (t + 1)],
                        start=True, stop=True,
                        tile_position=(32 * a, 0),
                    )
                ugs[i] = (p01, p23)
                if t == BLOCKS_PER_MT - 1:
                    del zpt[m]

            def relu(i):
                p01, p23 = ugs[i]
                v01 = vtp.tile([128, 1024], bf, tag="v01", name="v01")
                v23 = vtp.tile([128, 1024], bf, tag="v23", name="v23")
                if b1_zero:
                    nc.scalar.activation(
                        v01, p01, mybir.ActivationFunctionType.Relu)
                    nc.vector.tensor_scalar_max(v23, p23, 0.0)
                else:
                    nc.scalar.activation(
                        v01, p01, mybir.ActivationFunctionType.Relu,
                        bias=b1v, scale=1.0)
                    nc.vector.tensor_scalar(
                        v23, p23, b1v, 0.0,
                        mybir.AluOpType.add, mybir.AluOpType.max)
                vts[i] = (v01, v23)

            def mm3(i):
                v01, v23 = vts[i]
                ep = epp.tile([128, 512], f32, tag="ep", name="ep")
                for a in range(4):
                    rhs = v01 if a < 2 else v23
                    nc.tensor.matmul(
                        ep[32 * a:32 * (a + 1), :],
                        lhsT=wcst,
                        rhs=rhs[:, 512 * (a % 2):512 * (a % 2 + 1)],
                        start=True, stop=True,
                        tile_position=(0, 32 * a),
                    )
                eps_[i] = ep
                del ugs[i], vts[i]

            def eevac(i):
                m, t = divmod(i, BLOCKS_PER_MT)
                if t == 0:
                    ets[m] = etp.tile([128, 2048], edt, tag="et", name="et")
                et = ets[m]
                dst = et[:, 512 * t:512 * (t + 1)]
                src = eps_[i]
                # 5/8 of evacs on ACT: equalizes ACT (1059+5/8*582) and
                # DVE (1179+3/8*601) at ~1420ns/block
                if EEVAC == "act" or (EEVAC == "alt" and i % 8 < 5):
                    if b2_zero:
                        nc.scalar.activation(
                            dst, src, mybir.ActivationFunctionType.Identity)
                    else:
                        nc.scalar.activation(
                            dst, src, mybir.ActivationFunctionType.Identity,
                            bias=b2v, scale=1.0)
                else:
                    if b2_zero:
                        nc.vector.tensor_copy(dst, src)
                    else:
                        nc.vector.tensor_scalar_add(dst, src, b2v)
                del eps_[i]
                if m == NMT - 1:
                    # last megatile: per-block DMA so the output drain
                    # overlaps the final evacs instead of serializing
                    nc.sync.dma_start(out=EP[m][:, 512 * t:512 * (t + 1)],
                                      in_=dst)
                    if t == BLOCKS_PER_MT - 1:
                        del ets[m]
                elif t == BLOCKS_PER_MT - 1:
                    nc.sync.dma_start(out=EP[m], in_=et)
                    del ets[m]

            mm1(0)
            for i in range(NBLK):
                if i + 1 < NBLK:
                    mm1(i + 1)
                relu(i)
                mm3(i)
                if i > 0:
                    eevac(i - 1)
            eevac(NBLK - 1)

    nc.compile()
    _module_cache[key] = nc
    return nc


def _fold_params(matrix, W1, b1, W2, b2):
    """Host-side fold of the tiny params into A4/W2S/B1V/B2V (a few KB)."""
    matrix = np.asarray(matrix, np.float32)
    W1 = np.asarray(W1, np.float32)
    b1 = np.asarray(b1, np.float32)
    W2 = np.asarray(W2, np.float32)
    b2 = np.asarray(b2, np.float32)

    alpha_est = matrix * (matrix > np.float32(0.1)).astype(np.float32)
    mask = (alpha_est > np.float32(0.1)).astype(np.float32)  # (D, D)
    cnt = mask.sum(axis=1)  # (D,)
    scale = np.where(cnt > 0, np.float32(1.0) / np.maximum(cnt, 1.0),
                     np.float32(0.0)).astype(np.float32)
    M2 = (mask.T * scale[None, :]).astype(np.float32)  # M2[j,d]

    A = np.zeros((D, D * H), np.float32)
    for h in range(H):
        Ah = M2 * W1[None, :, 1, h]  # (j, d): M2[j,d] * W1[d,1,h]
        Ah[np.arange(D), np.arange(D)] += W1[:, 0, h]
        A[:, D * h:D * (h + 1)] = Ah
    A4 = np.ascontiguousarray(np.tile(A, (4, 1)))  # (128, 128)

    W2S = np.zeros((D * H, D), np.float32)
    W2S[np.arange(D * H), np.tile(np.arange(D), H)] = W2.T.reshape(-1)
    B1V = np.ascontiguousarray(b1.T.reshape(D * H, 1))
    B2V = np.ascontiguousarray(np.tile(b2, H).reshape(D * H, 1))
    return A4, W2S, B1V, B2V, not np.any(b1), not np.any(b2)


def _pack_z(Z):
    """(B, 32) f32 -> per-core (NMT, 128, 2048) bf16 strip layout:
    ZP[c][m, 32a+j, 512t+cc] = Z[c*R + m*8192 + t*2048 + a*512 + cc, j]."""
    Zb = np.asarray(Z, np.float32).astype(BF16)
    v = Zb.reshape(NCORES, NMT, 4, 4, 512, D)      # [c, m, t, a, cc, j]
    v = v.transpose(0, 1, 3, 5, 2, 4)              # [c, m, a, j, t, cc]
    return np.ascontiguousarray(v).reshape(NCORES, NMT, 128, 2048)


def _unpack_e(EPs):
    """per-core (NMT, 128, 2048) strip layout -> (B, 32) f32."""
    v = np.stack([np.asarray(e) for e in EPs])     # [c, m, 128, 2048]
    v = v.reshape(NCORES, NMT, 4, D, 4, 512)       # [c, m, a, d, t, cc]
    v = v.transpose(0, 1, 4, 2, 5, 3)              # [c, m, t, a, cc, d]
    return np.ascontiguousarray(v).reshape(B_TOTAL, D).astype(np.float32)


def _run(Z, matrix, W1, b1, W2, b2, trace=False):
    Z = np.asarray(Z, np.float32)
    assert Z.shape == (B_TOTAL, D), Z.shape
    A4, W2S, B1V, B2V, b1_zero, b2_zero = _fold_params(matrix, W1, b1, W2, b2)
    nc = _build_module(b1_zero, b2_zero)

    ZPall = _pack_z(Z)
    CB = np.concatenate([A4, W2S], axis=1).astype(BF16)  # (128, 160)
    CF = np.concatenate([B1V, B2V], axis=1).astype(np.float32)  # (128, 2)
    cst = {
        "CB": np.ascontiguousarray(CB),
        "CF": np.ascontiguousarray(CF),
    }
    in_maps = [{**cst, "ZP": ZPall[c]} for c in range(NCORES)]
    res = bass_utils.run_bass_kernel_spmd(
        nc, in_maps, core_ids=list(range(NCORES)), trace=trace)
    out = _unpack_e([r["EP"] for r in res.results])
    return out, res


def kernel(Z, matrix, W1, b1, W2, b2):
    out, _ = _run(Z, matrix, W1, b1, W2, b2, trace=False)
    return out


# revision 17
# speedup vs baseline: 1.0238x; 1.0238x over previous
"""Trainium2 Bass kernel for nn_CausalMultimodal (gnn_message_passing).

Math (per batch row b, fully row-local so batch shards freely over 8 cores):
    mask[i,j]  = (matrix*(matrix>0.1))[i,j] > 0.1
    agg[b,d]   = (Z[b,:] @ mask[d,:]) / count[d]   (0 when count==0)
    hidden     = relu(Z[b,d]*W1[d,0,h] + agg[b,d]*W1[d,1,h] + b1[d,h])
    E[b,d]     = sum_h hidden[b,d,h]*W2[d,h] + b2[d]

Since agg = Z @ M2 with M2[j,d] = mask[d,j]/count[d], the first layer folds
into one 32x128 matrix A computed host-side: U[b, 32h+d] = (Z @ A)[b, 32h+d];
then E = W2sel.T @ relu(U + b1) + b2 with W2sel (128,32) block-sparse.

v3 dataflow (PSUM-evacuation-bound; ACT+DVE are the critical engines):
  - Host pre-permutes Z (bf16) into the exact strip layout mm1 streams, and
    un-permutes the strip-layout E output. No DVE transposes on device.
  - Per 2048-row block: 4 row-tiled concurrent MMs (K=32, tile_position
    (32a,0)) write U into TWO (128,1024) PSUM pair-tiles (strips 0-1, 2-3).
    ACT relus pair01 while DVE relus pair23 (independent tiles, different
    banks). 4 col-tiled concurrent MMs (M=32, tile_position (0,32a)) write
    E into a separate (128,512) eps tile; E is evacuated PSUM->SBUF bf16 on
    alternating engines and DMA'd per megatile.
  - PSUM budget: 3 pair-slots x 2 banks + 2 eps banks = 8 banks. Every
    dependency edge has >= 1 block-period of slack (no aliasing), so the
    wall time tracks the busy-bound of the ACT/DVE engines.
  - Emission is software-pipelined: mm1 of block i+1 enters the PE FIFO
    before mm3 of block i; E-evac of block i is emitted one iteration late
    so it never head-of-line-blocks a relu in the strict-FIFO ACT/DVE
    queues.
"""

import os

import ml_dtypes
import numpy as np

import concourse.bacc as bacc
import concourse.tile as tile
from concourse import mybir
from concourse import bass_utils

B_TOTAL, D, H = 1048576, 32, 4
NCORES = 8
R = B_TOTAL // NCORES        # rows per core (131072)
NMT = 16                     # megatiles per core
BLOCKS_PER_MT = 4
NBLK = NMT * BLOCKS_PER_MT   # 64 blocks of 2048 rows
BF16 = ml_dtypes.bfloat16

EEVAC = os.environ.get("NNK_EEVAC", "alt")  # alt | act | dve
EOUT = os.environ.get("NNK_EOUT", "bf16")   # bf16 | f32
ZP_BUFS = int(os.environ.get("NNK_ZPBUFS", "3"))
PAIR_BUFS = int(os.environ.get("NNK_PAIRBUFS", "3"))
EPS_BUFS = int(os.environ.get("NNK_EPSBUFS", "2"))

_module_cache = {}


def _build_module(b1_zero, b2_zero):
    key = (b1_zero, b2_zero, EEVAC, EOUT, ZP_BUFS, PAIR_BUFS, EPS_BUFS)
    if key in _module_cache:
        return _module_cache[key]

    f32 = mybir.dt.float32
    bf = mybir.dt.bfloat16
    edt = bf if EOUT == "bf16" else f32

    nc = bacc.Bacc("TRN2", target_bir_lowering=False, debug=False,
                   num_devices=NCORES)

    ZP = nc.dram_tensor("ZP", (NMT, 128, 2048), bf, kind="ExternalInput").ap()
    # packed consts: CB[p] = [A4 row p (128 bf16) | W2S row p (32 bf16)]
    CB = nc.dram_tensor("CB", (128, 128 + D), bf, kind="ExternalInput").ap()
    # CF[p] = [b1v[p], b2v[p]]
    CF = nc.dram_tensor("CF", (128, 2), f32, kind="ExternalInput").ap()
    EP = nc.dram_tensor("EP", (NMT, 128, 2048), edt, kind="ExternalOutput").ap()

    with tile.TileContext(nc) as tc:
        with (
            tc.tile_pool(name="const", bufs=1) as constp,
            tc.tile_pool(name="zp", bufs=ZP_BUFS) as zpp,
            tc.tile_pool(name="vt", bufs=2) as vtp,
            tc.tile_pool(name="et", bufs=2) as etp,
            tc.tile_pool(name="ug", bufs=PAIR_BUFS, space="PSUM") as ugp,
            tc.tile_pool(name="ep", bufs=EPS_BUFS, space="PSUM") as epp,
        ):
            zpt = {}    # megatile -> SBUF tile
            ugs = {}    # block -> (pair01, pair23) PSUM tiles
            vts = {}    # block -> (vt01, vt23) SBUF tiles
            eps_ = {}   # block -> eps PSUM tile
            ets = {}    # megatile -> SBUF E tile

            def fetch_zp(m):
                t = zpp.tile([128, 2048], bf, tag="zp", name=f"zp{m}")
                nc.sync.dma_start(out=t, in_=ZP[m])
                zpt[m] = t

            # packed consts first (40KB, lands in ~0.5us), then the ZP
            # stream — mm1(0) needs the weights AND megatile 0.
            cbt = constp.tile([128, 128 + D], bf, name="cB")
            nc.sync.dma_start(out=cbt, in_=CB)
            cft = constp.tile([128, 2], f32, name="cF")
            nc.sync.dma_start(out=cft, in_=CF)
            acst = cbt[:, 0:128]
            wcst = cbt[:, 128:128 + D]
            b1v = cft[:, 0:1]
            b2v = cft[:, 1:2]
            for m in range(min(ZP_BUFS - 1, NMT)):
                fetch_zp(m)

            def mm1(i):
                m, t = divmod(i, BLOCKS_PER_MT)
                if t == 0 and m + ZP_BUFS - 1 < NMT:
                    fetch_zp(m + ZP_BUFS - 1)
                p01 = ugp.tile([128, 1024], f32, tag="ug", name="u01")
                p23 = ugp.tile([128, 1024], f32, tag="ug", name="u23")
                z = zpt[m]
                for a in range(4):
                    dst = p01 if a < 2 else p23
                    nc.tensor.matmul(
                        dst[:, 512 * (a % 2):512 * (a % 2 + 1)],
                        lhsT=acst[32 * a:32 * (a + 1), :],
                        rhs=z[32 * a:32 * (a + 1), 512 * t:512 * (t + 1)],
                        start=True, stop=True,
                        tile_position=(32 * a, 0),
                    )
                ugs[i] = (p01, p23)
                if t == BLOCKS_PER_MT - 1:
                    del zpt[m]

            def relu(i):
                p01, p23 = ugs[i]
                v01 = vtp.tile([128, 1024], bf, tag="v01", name="v01")
                v23 = vtp.tile([128, 1024], bf, tag="v23", name="v23")
                if b1_zero:
                    nc.scalar.activation(
                        v01, p01, mybir.ActivationFunctionType.Relu)
                    nc.vector.tensor_scalar_max(v23, p23, 0.0)
                else:
                    nc.scalar.activation(
                        v01, p01, mybir.ActivationFunctionType.Relu,
                        bias=b1v, scale=1.0)
                    nc.vector.tensor_scalar(
                        v23, p23, b1v, 0.0,
                        mybir.AluOpType.add, mybir.AluOpType.max)
                vts[i] = (v01, v23)

            def mm3(i):
                v01, v23 = vts[i]
                ep = epp.tile([128, 512], f32, tag="ep", name="ep")
                for a in range(4):
                    rhs = v01 if a < 2 else v23
                    nc.tensor.matmul(
                        ep[32 * a:32 * (a + 1), :],
                        lhsT=wcst,
                        rhs=rhs[:, 512 * (a % 2):512 * (a % 2 + 1)],
                        start=True, stop=True,
                        tile_position=(0, 32 * a),
                    )
                eps_[i] = ep
                del ugs[i], vts[i]

            def eevac(i):
                m, t = divmod(i, BLOCKS_PER_MT)
                if t == 0:
                    ets[m] = etp.tile([128, 2048], edt, tag="et", name="et")
                et = ets[m]
                dst = et[:, 512 * t:512 * (t + 1)]
                src = eps_[i]
                # 3/5 of evacs on ACT with max run length 2: equalizes
                # ACT (1051+0.6*618) and DVE (1180+0.4*602) at ~1420ns/block
                if EEVAC == "act" or (EEVAC == "alt" and i % 5 in (0, 2, 4)):
                    if b2_zero:
                        nc.scalar.activation(
                            dst, src, mybir.ActivationFunctionType.Identity)
                    else:
                        nc.scalar.activation(
                            dst, src, mybir.ActivationFunctionType.Identity,
                            bias=b2v, scale=1.0)
                else:
                    if b2_zero:
                        nc.vector.tensor_copy(dst, src)
                    else:
                        nc.vector.tensor_scalar_add(dst, src, b2v)
                del eps_[i]
                if m == NMT - 1:
                    # last megatile: per-block DMA so the output drain
                    # overlaps the final evacs instead of serializing
                    nc.sync.dma_start(out=EP[m][:, 512 * t:512 * (t + 1)],
                                      in_=dst)
                    if t == BLOCKS_PER_MT - 1:
                        del ets[m]
                elif t == BLOCKS_PER_MT - 1:
                    nc.sync.dma_start(out=EP[m], in_=et)
                    del ets[m]

            mm1(0)
            for i in range(NBLK):
                if i + 1 < NBLK:
                    mm1(i + 1)
                relu(i)
                mm3(i)
                if i > 0:
                    eevac(i - 1)
            eevac(NBLK - 1)

    nc.compile()
    _module_cache[key] = nc
    return nc


def _fold_params(matrix, W1, b1, W2, b2):
    """Host-side fold of the tiny params into A4/W2S/B1V/B2V (a few KB)."""
    matrix = np.asarray(matrix, np.float32)
    W1 = np.asarray(W1, np.float32)
    b1 = np.asarray(b1, np.float32)
    W2 = np.asarray(W2, np.float32)
    b2 = np.asarray(b2, np.float32)

    alpha_est = matrix * (matrix > np.float32(0.1)).astype(np.float32)
    mask = (alpha_est > np.float32(0.1)).astype(np.float32)  # (D, D)
    cnt = mask.sum(axis=1)  # (D,)
    scale = np.where(cnt > 0, np.float32(1.0) / np.maximum(cnt, 1.0),
                     np.float32(0.0)).astype(np.float32)
    M2 = (mask.T * scale[None, :]).astype(np.float32)  # M2[j,d]

    A = np.zeros((D, D * H), np.float32)
    for h in range(H):
        Ah = M2 * W1[None, :, 1, h]  # (j, d): M2[j,d] * W1[d,1,h]
        Ah[np.arange(D), np.arange(D)] += W1[:, 0, h]
        A[:, D * h:D * (h + 1)] = Ah
    A4 = np.ascontiguousarray(np.tile(A, (4, 1)))  # (128, 128)

    W2S = np.zeros((D * H, D), np.float32)
    W2S[np.arange(D * H), np.tile(np.arange(D), H)] = W2.T.reshape(-1)
    B1V = np.ascontiguousarray(b1.T.reshape(D * H, 1))
    B2V = np.ascontiguousarray(np.tile(b2, H).reshape(D * H, 1))
    return A4, W2S, B1V, B2V, not np.any(b1), not np.any(b2)


def _pack_z(Z):
    """(B, 32) f32 -> per-core (NMT, 128, 2048) bf16 strip layout:
    ZP[c][m, 32a+j, 512t+cc] = Z[c*R + m*8192 + t*2048 + a*512 + cc, j]."""
    Zb = np.asarray(Z, np.float32).astype(BF16)
    v = Zb.reshape(NCORES, NMT, 4, 4, 512, D)      # [c, m, t, a, cc, j]
    v = v.transpose(0, 1, 3, 5, 2, 4)              # [c, m, a, j, t, cc]
    return np.ascontiguousarray(v).reshape(NCORES, NMT, 128, 2048)


def _unpack_e(EPs):
    """per-core (NMT, 128, 2048) strip layout -> (B, 32) f32."""
    v = np.stack([np.asarray(e) for e in EPs])     # [c, m, 128, 2048]
    v = v.reshape(NCORES, NMT, 4, D, 4, 512)       # [c, m, a, d, t, cc]
    v = v.transpose(0, 1, 4, 2, 5, 3)              # [c, m, t, a, cc, d]
    return np.ascontiguousarray(v).reshape(B_TOTAL, D).astype(np.float32)


def _run(Z, matrix, W1, b1, W2, b2, trace=False):
    Z = np.asarray(Z, np.float32)
    assert Z.shape == (B_TOTAL, D), Z.shape
    A4, W2S, B1V, B2V, b1_zero, b2_zero = _fold_params(matrix, W1, b1, W2, b2)
    nc = _build_module(b1_zero, b2_zero)

    ZPall = _pack_z(Z)
    CB = np.concatenate([A4, W2S], axis=1).astype(BF16)  # (128, 160)
    CF = np.concatenate([B1V, B2V], axis=1).astype(np.float32)  # (128, 2)
    cst = {
        "CB": np.ascontiguousarray(CB),
        "CF": np.ascontiguousarray(CF),
    }
    in_maps = [{**cst, "ZP": ZPall[c]} for c in range(NCORES)]
    res = bass_utils.run_bass_kernel_spmd(
        nc, in_maps, core_ids=list(range(NCORES)), trace=trace)
    out = _unpack_e([r["EP"] for r in res.results])
    return out, res


def kernel(Z, matrix, W1, b1, W2, b2):
    out, _ = _run(Z, matrix, W1, b1, W2, b2, trace=False)
    return out


# revision 19
# speedup vs baseline: 1.0361x; 1.0120x over previous
"""Trainium2 Bass kernel for nn_CausalMultimodal (gnn_message_passing).

Math (per batch row b, fully row-local so batch shards freely over 8 cores):
    mask[i,j]  = (matrix*(matrix>0.1))[i,j] > 0.1
    agg[b,d]   = (Z[b,:] @ mask[d,:]) / count[d]   (0 when count==0)
    hidden     = relu(Z[b,d]*W1[d,0,h] + agg[b,d]*W1[d,1,h] + b1[d,h])
    E[b,d]     = sum_h hidden[b,d,h]*W2[d,h] + b2[d]

Since agg = Z @ M2 with M2[j,d] = mask[d,j]/count[d], the first layer folds
into one 32x128 matrix A computed host-side: U[b, 32h+d] = (Z @ A)[b, 32h+d];
then E = W2sel.T @ relu(U + b1) + b2 with W2sel (128,32) block-sparse.

v3 dataflow (PSUM-evacuation-bound; ACT+DVE are the critical engines):
  - Host pre-permutes Z (bf16) into the exact strip layout mm1 streams, and
    un-permutes the strip-layout E output. No DVE transposes on device.
  - Per 2048-row block: 4 row-tiled concurrent MMs (K=32, tile_position
    (32a,0)) write U into TWO (128,1024) PSUM pair-tiles (strips 0-1, 2-3).
    ACT relus pair01 while DVE relus pair23 (independent tiles, different
    banks). 4 col-tiled concurrent MMs (M=32, tile_position (0,32a)) write
    E into a separate (128,512) eps tile; E is evacuated PSUM->SBUF bf16 on
    alternating engines and DMA'd per megatile.
  - PSUM budget: 3 pair-slots x 2 banks + 2 eps banks = 8 banks. Every
    dependency edge has >= 1 block-period of slack (no aliasing), so the
    wall time tracks the busy-bound of the ACT/DVE engines.
  - Emission is software-pipelined: mm1 of block i+1 enters the PE FIFO
    before mm3 of block i; E-evac of block i is emitted one iteration late
    so it never head-of-line-blocks a relu in the strict-FIFO ACT/DVE
    queues.
"""

import os

import ml_dtypes
import numpy as np

import concourse.bacc as bacc
import concourse.tile as tile
from concourse import mybir
from concourse import bass_utils

B_TOTAL, D, H = 1048576, 32, 4
NCORES = 8
R = B_TOTAL // NCORES        # rows per core (131072)
NMT = 16                     # megatiles per core
BLOCKS_PER_MT = 4
NBLK = NMT * BLOCKS_PER_MT   # 64 blocks of 2048 rows
BF16 = ml_dtypes.bfloat16

EEVAC = os.environ.get("NNK_EEVAC", "alt")  # alt | act | dve
EOUT = os.environ.get("NNK_EOUT", "bf16")   # bf16 | f32
ZP_BUFS = int(os.environ.get("NNK_ZPBUFS", "3"))
PAIR_BUFS = int(os.environ.get("NNK_PAIRBUFS", "3"))
EPS_BUFS = int(os.environ.get("NNK_EPSBUFS", "2"))

_module_cache = {}


def _build_module(b1_zero, b2_zero):
    key = (b1_zero, b2_zero, EEVAC, EOUT, ZP_BUFS, PAIR_BUFS, EPS_BUFS)
    if key in _module_cache:
        return _module_cache[key]

    f32 = mybir.dt.float32
    bf = mybir.dt.bfloat16
    edt = bf if EOUT == "bf16" else f32

    nc = bacc.Bacc("TRN2", target_bir_lowering=False, debug=False,
                   num_devices=NCORES)

    ZP = nc.dram_tensor("ZP", (NMT, 128, 2048), bf, kind="ExternalInput").ap()
    # packed consts: CB[p] = [A4 row p (128 bf16) | W2S row p (32 bf16)]
    CB = nc.dram_tensor("CB", (128, 128 + D), bf, kind="ExternalInput").ap()
    # CF[p] = [b1v[p], b2v[p]]
    CF = nc.dram_tensor("CF", (128, 2), f32, kind="ExternalInput").ap()
    EP = nc.dram_tensor("EP", (NMT, 128, 2048), edt, kind="ExternalOutput").ap()

    with tile.TileContext(nc) as tc:
        with (
            tc.tile_pool(name="const", bufs=1) as constp,
            tc.tile_pool(name="zp", bufs=ZP_BUFS) as zpp,
            tc.tile_pool(name="vt", bufs=2) as vtp,
            tc.tile_pool(name="et", bufs=2) as etp,
            tc.tile_pool(name="ug", bufs=PAIR_BUFS, space="PSUM") as ugp,
            tc.tile_pool(name="ep", bufs=EPS_BUFS, space="PSUM") as epp,
        ):
            zpt = {}    # megatile -> SBUF tile
            ugs = {}    # block -> (pair01, pair23) PSUM tiles
            vts = {}    # block -> (vt01, vt23) SBUF tiles
            eps_ = {}   # block -> eps PSUM tile
            ets = {}    # megatile -> SBUF E tile

            def fetch_zp(m):
                t = zpp.tile([128, 2048], bf, tag="zp", name=f"zp{m}")
                # 4 chunk-DMAs: mm1 of block (m,t) waits only on chunk t,
                # so the first matmul isn't gated on the whole prefetch batch
                for tt in range(BLOCKS_PER_MT):
                    nc.sync.dma_start(out=t[:, 512 * tt:512 * (tt + 1)],
                                      in_=ZP[m][:, 512 * tt:512 * (tt + 1)])
                zpt[m] = t

            # packed consts first (40KB, lands in ~0.5us), then the ZP
            # stream — mm1(0) needs the weights AND megatile 0.
            cbt = constp.tile([128, 128 + D], bf, name="cB")
            nc.sync.dma_start(out=cbt, in_=CB)
            cft = constp.tile([128, 2], f32, name="cF")
            nc.sync.dma_start(out=cft, in_=CF)
            acst = cbt[:, 0:128]
            wcst = cbt[:, 128:128 + D]
            b1v = cft[:, 0:1]
            b2v = cft[:, 1:2]
            for m in range(min(ZP_BUFS - 1, NMT)):
                fetch_zp(m)

            def mm1(i):
                m, t = divmod(i, BLOCKS_PER_MT)
                if t == 0 and m + ZP_BUFS - 1 < NMT:
                    fetch_zp(m + ZP_BUFS - 1)
                p01 = ugp.tile([128, 1024], f32, tag="ug", name="u01")
                p23 = ugp.tile([128, 1024], f32, tag="ug", name="u23")
                z = zpt[m]
                for a in range(4):
                    dst = p01 if a < 2 else p23
                    nc.tensor.matmul(
                        dst[:, 512 * (a % 2):512 * (a % 2 + 1)],
                        lhsT=acst[32 * a:32 * (a + 1), :],
                        rhs=z[32 * a:32 * (a + 1), 512 * t:512 * (t + 1)],
                        start=True, stop=True,
                        tile_position=(32 * a, 0),
                    )
                ugs[i] = (p01, p23)
                if t == BLOCKS_PER_MT - 1:
                    del zpt[m]

            def relu(i):
                p01, p23 = ugs[i]
                v01 = vtp.tile([128, 1024], bf, tag="v01", name="v01")
                v23 = vtp.tile([128, 1024], bf, tag="v23", name="v23")
                if b1_zero:
                    nc.scalar.activation(
                        v01, p01, mybir.ActivationFunctionType.Relu)
                    nc.vector.tensor_scalar_max(v23, p23, 0.0)
                else:
                    nc.scalar.activation(
                        v01, p01, mybir.ActivationFunctionType.Relu,
                        bias=b1v, scale=1.0)
                    nc.vector.tensor_scalar(
                        v23, p23, b1v, 0.0,
                        mybir.AluOpType.add, mybir.AluOpType.max)
                vts[i] = (v01, v23)

            def mm3(i):
                v01, v23 = vts[i]
                ep = epp.tile([128, 512], f32, tag="ep", name="ep")
                for a in range(4):
                    rhs = v01 if a < 2 else v23
                    nc.tensor.matmul(
                        ep[32 * a:32 * (a + 1), :],
                        lhsT=wcst,
                        rhs=rhs[:, 512 * (a % 2):512 * (a % 2 + 1)],
                        start=True, stop=True,
                        tile_position=(0, 32 * a),
                    )
                eps_[i] = ep
                del ugs[i], vts[i]

            def eevac(i):
                m, t = divmod(i, BLOCKS_PER_MT)
                if t == 0:
                    ets[m] = etp.tile([128, 2048], edt, tag="et", name="et")
                et = ets[m]
                dst = et[:, 512 * t:512 * (t + 1)]
                src = eps_[i]
                # strict alternation: uneven ratios stall the pipeline more
                # than the ~60ns/block average gain is worth (measured)
                if EEVAC == "act" or (EEVAC == "alt" and i % 2 == 0):
                    if b2_zero:
                        nc.scalar.activation(
                            dst, src, mybir.ActivationFunctionType.Identity)
                    else:
                        nc.scalar.activation(
                            dst, src, mybir.ActivationFunctionType.Identity,
                            bias=b2v, scale=1.0)
                else:
                    if b2_zero:
                        nc.vector.tensor_copy(dst, src)
                    else:
                        nc.vector.tensor_scalar_add(dst, src, b2v)
                del eps_[i]
                if m == NMT - 1:
                    # last megatile: per-block DMA so the output drain
                    # overlaps the final evacs instead of serializing
                    nc.sync.dma_start(out=EP[m][:, 512 * t:512 * (t + 1)],
                                      in_=dst)
                    if t == BLOCKS_PER_MT - 1:
                        del ets[m]
                elif t == BLOCKS_PER_MT - 1:
                    nc.sync.dma_start(out=EP[m], in_=et)
                    del ets[m]

            mm1(0)
            for i in range(NBLK):
                if i + 1 < NBLK:
                    mm1(i + 1)
                relu(i)
                mm3(i)
                if i > 0:
                    eevac(i - 1)
            eevac(NBLK - 1)

    nc.compile()
    _module_cache[key] = nc
    return nc


def _fold_params(matrix, W1, b1, W2, b2):
    """Host-side fold of the tiny params into A4/W2S/B1V/B2V (a few KB)."""
    matrix = np.asarray(matrix, np.float32)
    W1 = np.asarray(W1, np.float32)
    b1 = np.asarray(b1, np.float32)
    W2 = np.asarray(W2, np.float32)
    b2 = np.asarray(b2, np.float32)

    alpha_est = matrix * (matrix > np.float32(0.1)).astype(np.float32)
    mask = (alpha_est > np.float32(0.1)).astype(np.float32)  # (D, D)
    cnt = mask.sum(axis=1)  # (D,)
    scale = np.where(cnt > 0, np.float32(1.0) / np.maximum(cnt, 1.0),
                     np.float32(0.0)).astype(np.float32)
    M2 = (mask.T * scale[None, :]).astype(np.float32)  # M2[j,d]

    A = np.zeros((D, D * H), np.float32)
    for h in range(H):
        Ah = M2 * W1[None, :, 1, h]  # (j, d): M2[j,d] * W1[d,1,h]
        Ah[np.arange(D), np.arange(D)] += W1[:, 0, h]
        A[:, D * h:D * (h + 1)] = Ah
    A4 = np.ascontiguousarray(np.tile(A, (4, 1)))  # (128, 128)

    W2S = np.zeros((D * H, D), np.float32)
    W2S[np.arange(D * H), np.tile(np.arange(D), H)] = W2.T.reshape(-1)
    B1V = np.ascontiguousarray(b1.T.reshape(D * H, 1))
    B2V = np.ascontiguousarray(np.tile(b2, H).reshape(D * H, 1))
    return A4, W2S, B1V, B2V, not np.any(b1), not np.any(b2)


def _pack_z(Z):
    """(B, 32) f32 -> per-core (NMT, 128, 2048) bf16 strip layout:
    ZP[c][m, 32a+j, 512t+cc] = Z[c*R + m*8192 + t*2048 + a*512 + cc, j]."""
    Zb = np.asarray(Z, np.float32).astype(BF16)
    v = Zb.reshape(NCORES, NMT, 4, 4, 512, D)      # [c, m, t, a, cc, j]
    v = v.transpose(0, 1, 3, 5, 2, 4)              # [c, m, a, j, t, cc]
    return np.ascontiguousarray(v).reshape(NCORES, NMT, 128, 2048)


def _unpack_e(EPs):
    """per-core (NMT, 128, 2048) strip layout -> (B, 32) f32."""
    v = np.stack([np.asarray(e) for e in EPs])     # [c, m, 128, 2048]
    v = v.reshape(NCORES, NMT, 4, D, 4, 512)       # [c, m, a, d, t, cc]
    v = v.transpose(0, 1, 4, 2, 5, 3)              # [c, m, t, a, cc, d]
    return np.ascontiguousarray(v).reshape(B_TOTAL, D).astype(np.float32)


def _run(Z, matrix, W1, b1, W2, b2, trace=False):
    Z = np.asarray(Z, np.float32)
    assert Z.shape == (B_TOTAL, D), Z.shape
    A4, W2S, B1V, B2V, b1_zero, b2_zero = _fold_params(matrix, W1, b1, W2, b2)
    nc = _build_module(b1_zero, b2_zero)

    ZPall = _pack_z(Z)
    CB = np.concatenate([A4, W2S], axis=1).astype(BF16)  # (128, 160)
    CF = np.concatenate([B1V, B2V], axis=1).astype(np.float32)  # (128, 2)
    cst = {
        "CB": np.ascontiguousarray(CB),
        "CF": np.ascontiguousarray(CF),
    }
    in_maps = [{**cst, "ZP": ZPall[c]} for c in range(NCORES)]
    res = bass_utils.run_bass_kernel_spmd(
        nc, in_maps, core_ids=list(range(NCORES)), trace=trace)
    out = _unpack_e([r["EP"] for r in res.results])
    return out, res


def kernel(Z, matrix, W1, b1, W2, b2):
    out, _ = _run(Z, matrix, W1, b1, W2, b2, trace=False)
    return out


# revision 23
# speedup vs baseline: 1.0494x; 1.0128x over previous
"""Trainium2 Bass kernel for nn_CausalMultimodal (gnn_message_passing).

Math (per batch row b, fully row-local so batch shards freely over 8 cores):
    mask[i,j]  = (matrix*(matrix>0.1))[i,j] > 0.1
    agg[b,d]   = (Z[b,:] @ mask[d,:]) / count[d]   (0 when count==0)
    hidden     = relu(Z[b,d]*W1[d,0,h] + agg[b,d]*W1[d,1,h] + b1[d,h])
    E[b,d]     = sum_h hidden[b,d,h]*W2[d,h] + b2[d]

Since agg = Z @ M2 with M2[j,d] = mask[d,j]/count[d], the first layer folds
into one 32x128 matrix A computed host-side: U[b, 32h+d] = (Z @ A)[b, 32h+d];
then E = W2sel.T @ relu(U + b1) + b2 with W2sel (128,32) block-sparse.

v3 dataflow (PSUM-evacuation-bound; ACT+DVE are the critical engines):
  - Host pre-permutes Z (bf16) into the exact strip layout mm1 streams, and
    un-permutes the strip-layout E output. No DVE transposes on device.
  - Per 2048-row block: 4 row-tiled concurrent MMs (K=32, tile_position
    (32a,0)) write U into TWO (128,1024) PSUM pair-tiles (strips 0-1, 2-3).
    ACT relus pair01 while DVE relus pair23 (independent tiles, different
    banks). 4 col-tiled concurrent MMs (M=32, tile_position (0,32a)) write
    E into a separate (128,512) eps tile; E is evacuated PSUM->SBUF bf16 on
    alternating engines and DMA'd per megatile.
  - PSUM budget: 3 pair-slots x 2 banks + 2 eps banks = 8 banks. Every
    dependency edge has >= 1 block-period of slack (no aliasing), so the
    wall time tracks the busy-bound of the ACT/DVE engines.
  - Emission is software-pipelined: mm1 of block i+1 enters the PE FIFO
    before mm3 of block i; E-evac of block i is emitted one iteration late
    so it never head-of-line-blocks a relu in the strict-FIFO ACT/DVE
    queues.
"""

import os

import ml_dtypes
import numpy as np

import concourse.bacc as bacc
import concourse.tile as tile
from concourse import mybir
from concourse import bass_utils

B_TOTAL, D, H = 1048576, 32, 4
NCORES = 8
R = B_TOTAL // NCORES        # rows per core (131072)
NMT = 16                     # megatiles per core
BLOCKS_PER_MT = 4
NBLK = NMT * BLOCKS_PER_MT   # 64 blocks of 2048 rows
BF16 = ml_dtypes.bfloat16

EEVAC = os.environ.get("NNK_EEVAC", "alt")  # alt | act | dve
EOUT = os.environ.get("NNK_EOUT", "bf16")   # bf16 | f32
ZP_BUFS = int(os.environ.get("NNK_ZPBUFS", "3"))
PAIR_BUFS = int(os.environ.get("NNK_PAIRBUFS", "3"))
EPS_BUFS = int(os.environ.get("NNK_EPSBUFS", "2"))

_module_cache = {}


def _build_module(b1_zero, b2_zero):
    key = (b1_zero, b2_zero, EEVAC, EOUT, ZP_BUFS, PAIR_BUFS, EPS_BUFS)
    if key in _module_cache:
        return _module_cache[key]

    f32 = mybir.dt.float32
    bf = mybir.dt.bfloat16
    edt = bf if EOUT == "bf16" else f32

    nc = bacc.Bacc("TRN2", target_bir_lowering=False, debug=False,
                   num_devices=NCORES)

    ZP = nc.dram_tensor("ZP", (NMT, 128, 2048), bf, kind="ExternalInput").ap()
    # packed consts: CB[p] = [A4 row p (128 bf16) | W2S row p (32 bf16)]
    CB = nc.dram_tensor("CB", (128, 128 + D), bf, kind="ExternalInput").ap()
    # CF[p] = [b1v[p], b2v[p]] — only materialized when a bias is nonzero
    CF = (None if b1_zero and b2_zero else
          nc.dram_tensor("CF", (128, 2), f32, kind="ExternalInput").ap())
    EP = nc.dram_tensor("EP", (NMT, 128, 2048), edt, kind="ExternalOutput").ap()

    with tile.TileContext(nc) as tc:
        with (
            tc.tile_pool(name="const", bufs=1) as constp,
            tc.tile_pool(name="zp", bufs=ZP_BUFS) as zpp,
            tc.tile_pool(name="vt", bufs=2) as vtp,
            tc.tile_pool(name="et", bufs=2) as etp,
            tc.tile_pool(name="ug", bufs=PAIR_BUFS, space="PSUM") as ugp,
            tc.tile_pool(name="ep", bufs=EPS_BUFS, space="PSUM") as epp,
        ):
            zpt = {}    # megatile -> SBUF tile
            ugs = {}    # block -> (pair01, pair23) PSUM tiles
            vts = {}    # block -> (vt01, vt23) SBUF tiles
            eps_ = {}   # block -> eps PSUM tile
            ets = {}    # megatile -> SBUF E tile

            def fetch_zp(m):
                t = zpp.tile([128, 2048], bf, tag="zp", name=f"zp{m}")
                if m == 0:
                    # chunk megatile 0 so mm1(0) waits only on a 128KB
                    # chunk, not the whole prefetch batch
                    for tt in range(BLOCKS_PER_MT):
                        nc.sync.dma_start(
                            out=t[:, 512 * tt:512 * (tt + 1)],
                            in_=ZP[m][:, 512 * tt:512 * (tt + 1)])
                else:
                    nc.sync.dma_start(out=t, in_=ZP[m])
                zpt[m] = t

            # DMA issue order drives the head: weights (40KB) first, then
            # megatile-0 chunks, bias vector (only if nonzero) and deeper
            # prefetch after — each dma_start costs ~0.6us of Sync issue
            # time, so the mm1(0) path must be first in the queue.
            cbt = constp.tile([128, 128 + D], bf, name="cB")
            nc.sync.dma_start(out=cbt, in_=CB)
            acst = cbt[:, 0:128]
            wcst = cbt[:, 128:128 + D]
            fetch_zp(0)
            if not (b1_zero and b2_zero):
                cft = constp.tile([128, 2], f32, name="cF")
                nc.sync.dma_start(out=cft, in_=CF)
                b1v = cft[:, 0:1]
                b2v = cft[:, 1:2]
            else:
                b1v = b2v = None
            for m in range(1, min(ZP_BUFS - 1, NMT)):
                fetch_zp(m)

            def mm1(i):
                m, t = divmod(i, BLOCKS_PER_MT)
                if t == 0 and m + ZP_BUFS - 1 < NMT:
                    fetch_zp(m + ZP_BUFS - 1)
                p01 = ugp.tile([128, 1024], f32, tag="ug", name="u01")
                p23 = ugp.tile([128, 1024], f32, tag="ug", name="u23")
                z = zpt[m]
                for a in range(4):
                    dst = p01 if a < 2 else p23
                    nc.tensor.matmul(
                        dst[:, 512 * (a % 2):512 * (a % 2 + 1)],
                        lhsT=acst[32 * a:32 * (a + 1), :],
                        rhs=z[32 * a:32 * (a + 1), 512 * t:512 * (t + 1)],
                        start=True, stop=True,
                        tile_position=(32 * a, 0),
                    )
                ugs[i] = (p01, p23)
                if t == BLOCKS_PER_MT - 1:
                    del zpt[m]

            def relu(i):
                p01, p23 = ugs[i]
                v01 = vtp.tile([128, 1024], bf, tag="v01", name="v01")
                v23 = vtp.tile([128, 1024], bf, tag="v23", name="v23")
                if b1_zero:
                    nc.scalar.activation(
                        v01, p01, mybir.ActivationFunctionType.Relu)
                    nc.vector.tensor_scalar_max(v23, p23, 0.0)
                else:
                    nc.scalar.activation(
                        v01, p01, mybir.ActivationFunctionType.Relu,
                        bias=b1v, scale=1.0)
                    nc.vector.tensor_scalar(
                        v23, p23, b1v, 0.0,
                        mybir.AluOpType.add, mybir.AluOpType.max)
                vts[i] = (v01, v23)

            def mm3(i):
                v01, v23 = vts[i]
                ep = epp.tile([128, 512], f32, tag="ep", name="ep")
                for a in range(4):
                    rhs = v01 if a < 2 else v23
                    nc.tensor.matmul(
                        ep[32 * a:32 * (a + 1), :],
                        lhsT=wcst,
                        rhs=rhs[:, 512 * (a % 2):512 * (a % 2 + 1)],
                        start=True, stop=True,
                        tile_position=(0, 32 * a),
                    )
                eps_[i] = ep
                del ugs[i], vts[i]

            def eevac(i):
                m, t = divmod(i, BLOCKS_PER_MT)
                if t == 0:
                    ets[m] = etp.tile([128, 2048], edt, tag="et", name="et")
                et = ets[m]
                dst = et[:, 512 * t:512 * (t + 1)]
                src = eps_[i]
                # strict alternation: uneven ratios stall the pipeline more
                # than the ~60ns/block average gain is worth (measured)
                if EEVAC == "act" or (EEVAC == "alt" and i % 2 == 0):
                    if b2_zero:
                        nc.scalar.activation(
                            dst, src, mybir.ActivationFunctionType.Identity)
                    else:
                        nc.scalar.activation(
                            dst, src, mybir.ActivationFunctionType.Identity,
                            bias=b2v, scale=1.0)
                else:
                    if b2_zero:
                        nc.vector.tensor_copy(dst, src)
                    else:
                        nc.vector.tensor_scalar_add(dst, src, b2v)
                del eps_[i]
                if m == NMT - 1:
                    # last megatile: per-block DMA so the output drain
                    # overlaps the final evacs instead of serializing
                    nc.sync.dma_start(out=EP[m][:, 512 * t:512 * (t + 1)],
                                      in_=dst)
                    if t == BLOCKS_PER_MT - 1:
                        del ets[m]
                elif t == BLOCKS_PER_MT - 1:
                    nc.sync.dma_start(out=EP[m], in_=et)
                    del ets[m]

            mm1(0)
            for i in range(NBLK):
                if i + 1 < NBLK:
                    mm1(i + 1)
                relu(i)
                mm3(i)
                if i > 0:
                    eevac(i - 1)
            eevac(NBLK - 1)

    nc.compile()
    _module_cache[key] = nc
    return nc


def _fold_params(matrix, W1, b1, W2, b2):
    """Host-side fold of the tiny params into A4/W2S/B1V/B2V (a few KB)."""
    matrix = np.asarray(matrix, np.float32)
    W1 = np.asarray(W1, np.float32)
    b1 = np.asarray(b1, np.float32)
    W2 = np.asarray(W2, np.float32)
    b2 = np.asarray(b2, np.float32)

    alpha_est = matrix * (matrix > np.float32(0.1)).astype(np.float32)
    mask = (alpha_est > np.float32(0.1)).astype(np.float32)  # (D, D)
    cnt = mask.sum(axis=1)  # (D,)
    scale = np.where(cnt > 0, np.float32(1.0) / np.maximum(cnt, 1.0),
                     np.float32(0.0)).astype(np.float32)
    M2 = (mask.T * scale[None, :]).astype(np.float32)  # M2[j,d]

    A = np.zeros((D, D * H), np.float32)
    for h in range(H):
        Ah = M2 * W1[None, :, 1, h]  # (j, d): M2[j,d] * W1[d,1,h]
        Ah[np.arange(D), np.arange(D)] += W1[:, 0, h]
        A[:, D * h:D * (h + 1)] = Ah
    A4 = np.ascontiguousarray(np.tile(A, (4, 1)))  # (128, 128)

    W2S = np.zeros((D * H, D), np.float32)
    W2S[np.arange(D * H), np.tile(np.arange(D), H)] = W2.T.reshape(-1)
    B1V = np.ascontiguousarray(b1.T.reshape(D * H, 1))
    B2V = np.ascontiguousarray(np.tile(b2, H).reshape(D * H, 1))
    return A4, W2S, B1V, B2V, not np.any(b1), not np.any(b2)


def _pack_z(Z):
    """(B, 32) f32 -> per-core (NMT, 128, 2048) bf16 strip layout:
    ZP[c][m, 32a+j, 512t+cc] = Z[c*R + m*8192 + t*2048 + a*512 + cc, j]."""
    Zb = np.asarray(Z, np.float32).astype(BF16)
    v = Zb.reshape(NCORES, NMT, 4, 4, 512, D)      # [c, m, t, a, cc, j]
    v = v.transpose(0, 1, 3, 5, 2, 4)              # [c, m, a, j, t, cc]
    return np.ascontiguousarray(v).reshape(NCORES, NMT, 128, 2048)


def _unpack_e(EPs):
    """per-core (NMT, 128, 2048) strip layout -> (B, 32) f32."""
    v = np.stack([np.asarray(e) for e in EPs])     # [c, m, 128, 2048]
    v = v.reshape(NCORES, NMT, 4, D, 4, 512)       # [c, m, a, d, t, cc]
    v = v.transpose(0, 1, 4, 2, 5, 3)              # [c, m, t, a, cc, d]
    return np.ascontiguousarray(v).reshape(B_TOTAL, D).astype(np.float32)


def _run(Z, matrix, W1, b1, W2, b2, trace=False):
    Z = np.asarray(Z, np.float32)
    assert Z.shape == (B_TOTAL, D), Z.shape
    A4, W2S, B1V, B2V, b1_zero, b2_zero = _fold_params(matrix, W1, b1, W2, b2)
    nc = _build_module(b1_zero, b2_zero)

    ZPall = _pack_z(Z)
    CB = np.concatenate([A4, W2S], axis=1).astype(BF16)  # (128, 160)
    cst = {"CB": np.ascontiguousarray(CB)}
    if not (b1_zero and b2_zero):
        CF = np.concatenate([B1V, B2V], axis=1).astype(np.float32)
        cst["CF"] = np.ascontiguousarray(CF)
    in_maps = [{**cst, "ZP": ZPall[c]} for c in range(NCORES)]
    res = bass_utils.run_bass_kernel_spmd(
        nc, in_maps, core_ids=list(range(NCORES)), trace=trace)
    out = _unpack_e([r["EP"] for r in res.results])
    return out, res


def kernel(Z, matrix, W1, b1, W2, b2):
    out, _ = _run(Z, matrix, W1, b1, W2, b2, trace=False)
    return out
